# revision 1
# baseline (speedup 1.0000x reference)
"""Trainium2 Bass kernel for ConstrainedAttentionModel (sparse_attention).

Full-input contract: kernel(x=[8,2048] int, C=[4,4] f32) -> [8,2048] f32.
Data parallel across 8 NeuronCores: one batch row per core.

Math (per row, T=2048, k=4, V=2048):
  scores[t] = sum_{i,j} C[i,j] * [x[t-j] == x[T-1-i]]   (t-j >= 0)
  scores[T-1] = -1e9; attn = softmax(scores)
  out[v] = sum_t attn[t] * [x[t] == v]

Device strategy (t = 16p + f layout on 128 partitions):
  - one contiguous DMA loads a 19-token window per partition; the 4
    shifted copies are overlapping SBUF *views*
  - the tiny const row (queries, hi/lo iotas, C) lands on one partition
    and is broadcast to all 128 via a K=1 PE matmul (avoids the slow
    replicated-read DMA)
  - warm-up matmuls keep the PE HAM clock gate open so the real
    contraction runs at full rate
  - equality against the 4 query tokens, weighted by C, reduced ->
    scores; softmax mask folded in as a 17th reduce channel from HBM
  - exp on the scalar engine with fused row-sum accumulation
  - vocab one-hot factorized as v = 64*hi + lo: A[t,hi]=E[t]*[x>>6==hi],
    B[t,lo]=[x&63==lo]; out[hi,lo] = A^T@B as 16 PSUM-accumulated
    matmuls, split in f-halves to overlap DVE and PE
  - 1/sum(E) via ones-matmul + reciprocal + ones-broadcast on PE,
    applied by DVE reading PSUM directly
"""
import os
import numpy as np
import concourse.bass as bass
import concourse.bacc as bacc
import concourse.tile as tile
from concourse import mybir

P = 128
T = 2048
F = T // P  # 16
K = 4
FH = F // 2  # 8
NHI = 32
NLO = 64
NEG = -1.0e9
XW = F + K - 1  # 19

fp32 = mybir.dt.float32
i32 = mybir.dt.int32
Alu = mybir.AluOpType
Act = mybir.ActivationFunctionType

# const row (fp32 values): q, iota_hi (64*i), iota_lo, C
CV_Q = 0
CV_IH = 16
CV_IL = 48
CV_QI = 112  # fp16 words of q+iotas
CV_CB = 64  # fp32 word offset of C block in packed crow
CV_QI_W = 56  # fp32 words holding the 112 fp16 values
CV_LEN = 80

N_WARM1 = int(os.environ.get("KERNEL_N_WARM1", "14"))
N_WARM2 = int(os.environ.get("KERNEL_N_WARM2", "22"))


B = 8


def _build_nc():
    nc = bacc.Bacc()
    xin = nc.dram_tensor("xin", [K - 1 + T], i32, kind="ExternalInput")
    crow = nc.dram_tensor("crow", [CV_LEN], fp32, kind="ExternalInput")
    bvec = nc.dram_tensor("bvec", [T], fp32, kind="ExternalInput")
    y = nc.dram_tensor("y", [T], fp32, kind="ExternalOutput")

    with tile.TileContext(nc) as tc:
        with (
            tc.tile_pool(name="sb", bufs=1) as sb,
            tc.tile_pool(name="ps", bufs=1, space="PSUM") as ps,
        ):
            XF = sb.tile([P, XW], i32)  # XF[p,e] = x[16p+e-3], pad -1
            CROW = sb.tile([1, CV_LEN], fp32)  # [0:64]: fp16-packed q+iotas
            CE = sb.tile([P, F, 17], fp32)  # c<16 products, c=16 mask bias

            nc.sync.dma_start(
                out=XF[:],
                in_=bass.AP(tensor=xin[:].tensor, offset=0, ap=[[F, P], [1, XW]]),
            )
            nc.scalar.dma_start(out=CROW[:], in_=crow[None, :])
            nc.sync.dma_start(
                out=CE[:, :, 16],
                in_=bass.AP(tensor=bvec[:].tensor, offset=0, ap=[[F, P], [1, F]]),
            )

            # ones rows (no deps -> run immediately) + dummy rhs for warmup
            ONESB = sb.tile([1, P], fp32)
            nc.vector.memset(ONESB[:], 1.0)
            ONESH = sb.tile([1, P], mybir.dt.float16)
            nc.vector.memset(ONESH[:], 1.0)
            c1 = nc.const_aps.aps[(fp32, 1.0)]

            # PE warm-up: narrow matmuls keep the HAM clock gate open
            warm = ps.tile([1, 1], fp32)
            for w in range(N_WARM1):
                nc.tensor.matmul(
                    warm[:], lhsT=c1[:, 0:1], rhs=c1[:, 0:1], start=True,
                    stop=True, skip_group_check=True,
                )

            # broadcast const row to all partitions via K=1 matmuls:
            # C (fp32) first -- it gates the CE multiply on the critical path
            CBCC = ps.tile([P, 16], fp32)
            nc.tensor.matmul(
                CBCC[:], lhsT=ONESB[:], rhs=CROW[:, CV_CB : CV_CB + 16],
                start=True, stop=True, skip_group_check=True,
            )
            CBC = ps.tile([P, CV_QI], fp32)
            nc.tensor.matmul(
                CBC[:],
                lhsT=ONESH[:],
                rhs=CROW[:, 0:CV_QI_W].bitcast(mybir.dt.float16)[:, 0:CV_QI],
                start=True,
                stop=True,
                skip_group_check=True,
            )

            for w in range(N_WARM2):
                nc.tensor.matmul(
                    warm[:], lhsT=c1[:, 0:1], rhs=c1[:, 0:1], start=True,
                    stop=True, skip_group_check=True,
                )

            CN = CBCC[:, 0:16]

            # integer copy of q + iotas so all compares stay int32
            CBI = sb.tile([P, CV_QI], i32)
            with tc.high_priority():
                nc.vector.tensor_copy(out=CBI[:], in_=CBC[:])
            QV4 = CBI[:, CV_Q : CV_Q + 16].rearrange("p (j i) -> p j i", i=K)
            IH = CBI[:, CV_IH : CV_IH + NHI]
            IL = CBI[:, CV_IL : CV_IL + NLO]

            # x & 63 / x & ~63 for the lo/hi one-hots
            X0 = XF[:, K - 1 : K - 1 + F]
            XLH = sb.tile([P, 2 * F], i32)
            nc.vector.tensor_scalar(
                out=XLH[:, 0:F], in0=X0, scalar1=63, scalar2=None,
                op0=Alu.bitwise_and,
            )
            nc.vector.tensor_scalar(
                out=XLH[:, F : 2 * F], in0=X0, scalar1=-64, scalar2=None,
                op0=Alu.bitwise_and,
            )
            XLO = XLH[:, 0:F]
            XH64 = XLH[:, F : 2 * F]

            EQ = sb.tile([P, F, K, K], fp32)
            SC = sb.tile([P, F], fp32)
            E = sb.tile([P, F], fp32)
            RS = sb.tile([P, 2], fp32)
            AEQ = sb.tile([P, F, NHI], fp32)
            Bt = sb.tile([P, F, NLO], fp32)
            A = sb.tile([P, F, NHI], fp32)
            acc = ps.tile([NHI, NLO], fp32)
            S1 = ps.tile([1, 2], fp32)
            RINV = sb.tile([1, 1], fp32)
            RB = ps.tile([NHI, 1], fp32)

            # two fully pipelined f-half chains
            for h in range(2):
                fs = slice(h * FH, (h + 1) * FH)
                sub = XF[:, h * FH : h * FH + FH + K - 1][:]
                XWIN = bass.AP(
                    tensor=sub.tensor,
                    offset=sub.offset,
                    ap=[sub.ap[0], [1, FH], [1, K], [0, K]],
                )  # [P, FH, jj, i] = x[t-(3-jj)] int32
                with tc.high_priority():
                    nc.vector.tensor_tensor(
                        out=EQ[:, fs],
                        in0=XWIN,
                        in1=QV4[:, None, :, :].broadcast_to([P, FH, K, K]),
                        op=Alu.is_equal,
                    )
                    nc.vector.tensor_tensor(
                        out=CE[:, fs, 0:16],
                        in0=EQ[:, fs].rearrange("p f j i -> p f (j i)"),
                        in1=CN[:, None, :].broadcast_to([P, FH, 16]),
                        op=Alu.mult,
                    )
                    nc.vector.reduce_sum(
                        out=SC[:, fs], in_=CE[:, fs], axis=mybir.AxisListType.X
                    )
                nc.scalar.activation(
                    out=E[:, fs], in_=SC[:, fs], func=Act.Exp,
                    accum_out=RS[:, h : h + 1],
                )
                nc.vector.tensor_tensor(
                    out=AEQ[:, fs],
                    in0=XH64[:, fs][:, :, None].broadcast_to([P, FH, NHI]),
                    in1=IH[:, None, :].broadcast_to([P, FH, NHI]),
                    op=Alu.is_equal,
                )
                nc.vector.tensor_tensor(
                    out=Bt[:, fs],
                    in0=XLO[:, fs][:, :, None].broadcast_to([P, FH, NLO]),
                    in1=IL[:, None, :].broadcast_to([P, FH, NLO]),
                    op=Alu.is_equal,
                )
                with tc.high_priority():
                    nc.vector.tensor_tensor(
                        out=A[:, fs],
                        in0=AEQ[:, fs],
                        in1=E[:, fs][:, :, None].broadcast_to([P, FH, NHI]),
                        op=Alu.mult,
                    )
                for f in range(h * FH, (h + 1) * FH):
                    nc.tensor.matmul(
                        acc[:],
                        lhsT=A[:, f, :],
                        rhs=Bt[:, f, :],
                        start=(f == 0),
                        stop=(f == F - 1),
                        skip_group_check=True,
                    )
            nc.tensor.matmul(
                S1[:], lhsT=c1[:, 0:1], rhs=RS[:], start=True,
                stop=True, skip_group_check=True,
            )
            SS = sb.tile([1, 1], fp32)
            nc.vector.reduce_sum(out=SS[:], in_=S1[:], axis=mybir.AxisListType.X)
            nc.vector.reciprocal(out=RINV[:], in_=SS[:])
            nc.tensor.matmul(
                RB[:], lhsT=ONESB[0:1, 0:NHI], rhs=RINV[:], start=True,
                stop=True, skip_group_check=True,
            )

            OUT = sb.tile([NHI, NLO], fp32)
            nc.vector.tensor_scalar(
                out=OUT[:], in0=acc[:], scalar1=RB[:], scalar2=None, op0=Alu.mult
            )
            nc.sync.dma_start(out=y[:].rearrange("(h l) -> h l", l=NLO), in_=OUT[:])
    nc.compile()
    return nc




def _make_crow(x_row: np.ndarray, C: np.ndarray) -> np.ndarray:
    qi = np.zeros(2 * CV_QI_W, np.float16)  # fp16 block (112 used)
    q = x_row[T - 1 : T - 1 - K : -1].astype(np.float16)  # q[i] = x[T-1-i]
    qi[CV_Q : CV_Q + 16] = np.tile(q, K)
    qi[CV_IH : CV_IH + NHI] = 64.0 * np.arange(NHI, dtype=np.float16)
    qi[CV_IL : CV_IL + NLO] = np.arange(NLO, dtype=np.float16)
    cv = np.zeros(CV_LEN, np.float32)
    cv[0:CV_QI_W] = qi.view(np.float32)
    # crow[CV_CB + jj*4+i] = C[i, 3-jj]
    cv[CV_CB : CV_CB + 16] = (
        np.ascontiguousarray(C[:, ::-1].T).reshape(16).astype(np.float32)
    )
    return cv




def _host_prep(x_row: np.ndarray, C: np.ndarray):
    x_row = x_row.astype(np.int32)
    xin = np.concatenate([np.full(K - 1, -1, np.int32), x_row])
    bvec = np.zeros(T, np.float32)
    bvec[T - 1] = NEG
    return {"xin": xin, "crow": _make_crow(x_row, C), "bvec": bvec}




_NC_CACHE = {}


def _get_nc():
    if "nc" not in _NC_CACHE:
        _NC_CACHE["nc"] = _build_nc()
    return _NC_CACHE["nc"]


def kernel(x: np.ndarray, C: np.ndarray, _spmd_kwargs: dict | None = None):
    from concourse.bass_utils import run_bass_kernel_spmd

    x = np.asarray(x).astype(np.int32)  # token ids < 2048, exact
    C = np.asarray(C).astype(np.float32)
    assert x.shape == (B, T) and C.shape == (K, K)
    in_maps = [_host_prep(x[b], C) for b in range(B)]
    res = run_bass_kernel_spmd(
        _get_nc(), in_maps, core_ids=list(range(B)), **(_spmd_kwargs or {})
    )
    out = np.stack([res.results[b]["y"] for b in range(B)], axis=0)
    if _spmd_kwargs:
        kernel.last_results = res
    return out



# revision 2
# speedup vs baseline: 1.2782x; 1.2782x over previous
"""Trainium2 Bass kernel for ConstrainedAttentionModel (sparse_attention).

Full-input contract: kernel(x=[8,2048] int, C=[4,4] f32) -> [8,2048] f32.
Data parallel across 8 NeuronCores: one batch row per core.

Math (per row, T=2048, k=4, V=2048):
  scores[t] = sum_{i,j} C[i,j] * [x[t-j] == x[T-1-i]]   (t-j >= 0)
  scores[T-1] = -inf; attn = softmax(scores)
  out[v] = sum_t attn[t] * [x[t] == v]

v2 design (t = 16p + f layout on 128 partitions):
  - ONE input DMA: the host packs a per-partition image holding the
    fp16 x-window (20 wide), queries replicated across the window,
    C (re-ordered for the conv view), the softmax-mask bias row,
    the base-64 digits of x (lo=x&63, hi=x>>6), the class iotas and
    ones rows -- no const-broadcast matmuls, no casts, no extra DMAs
  - windowed equality m[p,i,e] = [x_win[p,e]==q_i] (76 useful elems,
    fp16 2x mode), conv view with C -> scores, mask folded in as a
    17th reduce channel copied from the image by the scalar engine
  - exp on the scalar engine with fused row-sum accumulation
  - vocab one-hot factorized v = 64*hi + lo in fp16; out[hi,lo] =
    sum_f A_f^T @ B_f as 16 fp16 PSUM-accumulated matmuls (fp32 PE
    matmuls are 4x slower)
  - 1/sum(E) via ones-matmul + reciprocal + ones-broadcast on PE,
    applied by DVE reading PSUM directly
  - instruction count kept minimal: the framework's per-semaphore
    teardown and instruction-queue loads dominate at this scale
"""
import numpy as np
import concourse.bass as bass
import concourse.bacc as bacc
import concourse.tile as tile
from concourse import mybir

P = 128
T = 2048
F = T // P  # 16
K = 4
FH = F // 2  # 8
NHI = 32
NLO = 64
WIN = 20  # x-window width per partition (19 used, padded to 20)
NEG = -60000.0  # large-negative mask bias, exactly representable in fp16

fp32 = mybir.dt.float32
fp16 = mybir.dt.float16
i32 = mybir.dt.int32
Alu = mybir.AluOpType
Act = mybir.ActivationFunctionType

# int32-word offsets inside the packed per-partition image
OFF_XW = 0  # [20] fp16 x-window          -> 10 words
OFF_QR = 10  # [4,20] fp16 query replicas   -> 40 words
OFF_CR = 50  # [16] fp16 C (i,jj) order     ->  8 words
OFF_BIAS = 58  # [16] fp16 mask bias row      ->  8 words
OFF_XLO = 66  # [16] fp16 x & 63             ->  8 words
OFF_XHI = 74  # [16] fp16 x >> 6             ->  8 words
OFF_IL = 82  # [64] fp16 iota 0..63         -> 32 words
OFF_IH = 114  # [32] fp16 iota 0..31         -> 16 words
OFF_ONE = 130  # [1] fp32 ones column         ->  1 word
OFF_ONR = 131  # [32] fp32 ones row           -> 32 words
IMG_W = 163

B = 8


def _build_nc():
    nc = bacc.Bacc()
    img = nc.dram_tensor("img", [P, IMG_W], i32, kind="ExternalInput")
    y = nc.dram_tensor("y", [T], fp32, kind="ExternalOutput")

    with tile.TileContext(nc) as tc:
        with (
            tc.tile_pool(name="sb", bufs=1) as sb,
            tc.tile_pool(name="ps", bufs=1, space="PSUM") as ps,
        ):
            IMGT = sb.tile([P, IMG_W], i32)
            nc.sync.dma_start(out=IMGT[:], in_=img[:])

            xw = IMGT[:, OFF_XW : OFF_XW + 10].bitcast(fp16)  # [P, 20]
            qr = IMGT[:, OFF_QR : OFF_QR + 40].bitcast(fp16).rearrange(
                "p (i e) -> p i e", e=WIN
            )  # [P, 4, 20]
            cr = IMGT[:, OFF_CR : OFF_CR + 8].bitcast(fp16).rearrange(
                "p (i jj) -> p i jj", jj=K
            )  # [P, 4, 4]
            bias = IMGT[:, OFF_BIAS : OFF_BIAS + 8].bitcast(fp16)  # [P, 16]
            xlo = IMGT[:, OFF_XLO : OFF_XLO + 8].bitcast(fp16)  # [P, 16]
            xhi = IMGT[:, OFF_XHI : OFF_XHI + 8].bitcast(fp16)  # [P, 16]
            il = IMGT[:, OFF_IL : OFF_IL + 32].bitcast(fp16)  # [P, 64]
            ih = IMGT[:, OFF_IH : OFF_IH + 16].bitcast(fp16)  # [P, 32]
            onec = IMGT[:, OFF_ONE : OFF_ONE + 1].bitcast(fp32)  # [P, 1]
            oner = IMGT[:, OFF_ONR : OFF_ONR + 32].bitcast(fp32)  # [P, 32]

            EQ = sb.tile([P, K, WIN], fp16)  # m[p,i,e] = [xw[p,e]==q_i]
            CE = sb.tile([P, F, 17], fp16)  # c<16: C*m products, c=16: bias
            SC = sb.tile([P, F], fp32)
            E = sb.tile([P, F], fp16)
            RS = sb.tile([P, 1], fp32)
            AEQ = sb.tile([P, F, NHI], fp16)
            BT = sb.tile([P, F, NLO], fp16)
            A = sb.tile([P, F, NHI], fp16)
            RINV = sb.tile([1, 1], fp32)
            OUT = sb.tile([NHI, NLO], fp32)
            acc = ps.tile([NHI, NLO], fp32)
            S1 = ps.tile([1, 1], fp32)
            RB = ps.tile([NHI, 1], fp32)

            # mask bias -> 17th reduce channel (scalar engine, off critical path)
            nc.scalar.activation(out=CE[:, :, 16], in_=bias, func=Act.Copy)

            # m[p,i,e] = [x[16p+e-3] == q_i]  (fp16, all inner steps 1 -> 2x)
            nc.vector.tensor_tensor(
                out=EQ[:],
                in0=xw[:, None, :].broadcast_to([P, K, WIN]),
                in1=qr,
                op=Alu.is_equal,
            )
            # CE[p,f,(i,jj)] = C[i,3-jj] * m[p,i,f+jj]
            eq = EQ[:]
            EQV = bass.AP(
                tensor=eq.tensor,
                offset=eq.offset,
                ap=[eq.ap[0], [1, F], [WIN, K], [1, K]],
            )  # [P, f, i, jj] = m[p, i, f+jj]
            nc.vector.tensor_tensor(
                out=CE[:, :, 0:16].rearrange("p f (i jj) -> p f i jj", jj=K),
                in0=EQV,
                in1=cr[:, None, :, :].broadcast_to([P, F, K, K]),
                op=Alu.mult,
            )
            # scores + mask bias
            nc.vector.reduce_sum(out=SC[:], in_=CE[:], axis=mybir.AxisListType.X)
            # E = exp(scores), RS = per-partition sum of E
            nc.scalar.activation(
                out=E[:], in_=SC[:], func=Act.Exp, accum_out=RS[:]
            )
            # S1 = sum_p RS  (PE; waits on RS, then queue runs the acc matmuls)
            nc.tensor.matmul(
                S1[:], lhsT=onec, rhs=RS[:], start=True, stop=True,
                skip_group_check=True,
            )

            # one-hot build + E-weighting + PSUM-accumulated outer products,
            # split in f-halves so PE h0 overlaps DVE h1
            for h in range(2):
                fs = slice(h * FH, (h + 1) * FH)
                nc.vector.tensor_tensor(
                    out=AEQ[:, fs],
                    in0=xhi[:, fs][:, :, None].broadcast_to([P, FH, NHI]),
                    in1=ih[:, None, :].broadcast_to([P, FH, NHI]),
                    op=Alu.is_equal,
                )
                nc.vector.tensor_tensor(
                    out=BT[:, fs],
                    in0=xlo[:, fs][:, :, None].broadcast_to([P, FH, NLO]),
                    in1=il[:, None, :].broadcast_to([P, FH, NLO]),
                    op=Alu.is_equal,
                )
                nc.vector.tensor_tensor(
                    out=A[:, fs],
                    in0=AEQ[:, fs],
                    in1=E[:, fs][:, :, None].broadcast_to([P, FH, NHI]),
                    op=Alu.mult,
                )
                for f in range(h * FH, (h + 1) * FH):
                    nc.tensor.matmul(
                        acc[:],
                        lhsT=A[:, f, :],
                        rhs=BT[:, f, :],
                        start=(f == 0),
                        stop=(f == F - 1),
                        skip_group_check=True,
                    )

            # 1/S broadcast to the 32 output partitions
            nc.vector.reciprocal(out=RINV[:], in_=S1[:])
            nc.tensor.matmul(
                RB[:], lhsT=oner[0:1, :], rhs=RINV[:], start=True, stop=True,
                skip_group_check=True,
            )
            nc.vector.tensor_scalar(
                out=OUT[:], in0=acc[:], scalar1=RB[:], scalar2=None, op0=Alu.mult
            )
            nc.sync.dma_start(
                out=y[:].rearrange("(h l) -> h l", l=NLO), in_=OUT[:]
            )
    nc.compile()
    return nc


def _host_prep(x_row: np.ndarray, C: np.ndarray):
    x_row = x_row.astype(np.int32)
    xpad = np.concatenate(
        [np.full(K - 1, -1, np.int32), x_row, np.full(1, -1, np.int32)]
    )
    idx = 16 * np.arange(P)[:, None] + np.arange(WIN)[None, :]
    xw = xpad[idx].astype(np.float16)  # [128, 20]
    q = x_row[T - 1 : T - 1 - K : -1].astype(np.float16)  # q[i] = x[T-1-i]
    qrep = np.tile(q[:, None], (1, WIN)).reshape(-1)  # [80]
    cr = np.ascontiguousarray(C[:, ::-1]).astype(np.float16).reshape(-1)  # [16]
    bias = np.zeros((P, F), np.float16)
    bias[P - 1, F - 1] = NEG
    xt = x_row.reshape(P, F)
    xlo = (xt & 63).astype(np.float16)
    xhi = (xt >> 6).astype(np.float16)
    il = np.arange(NLO, dtype=np.float16)
    ih = np.arange(NHI, dtype=np.float16)
    onec = np.ones(1, np.float32)
    oner = np.ones(NHI, np.float32)

    img = np.empty((P, IMG_W * 4), np.uint8)
    for p in range(P):
        row = np.concatenate(
            [
                xw[p].view(np.uint8),
                qrep.view(np.uint8),
                cr.view(np.uint8),
                bias[p].view(np.uint8),
                xlo[p].view(np.uint8),
                xhi[p].view(np.uint8),
                il.view(np.uint8),
                ih.view(np.uint8),
                onec.view(np.uint8),
                oner.view(np.uint8),
            ]
        )
        img[p] = row
    return {"img": img.view(np.int32)}


_NC_CACHE = {}


def _get_nc():
    if "nc" not in _NC_CACHE:
        _NC_CACHE["nc"] = _build_nc()
    return _NC_CACHE["nc"]


def kernel(x: np.ndarray, C: np.ndarray, _spmd_kwargs: dict | None = None):
    from concourse.bass_utils import run_bass_kernel_spmd

    x = np.asarray(x).astype(np.int32)  # token ids < 2048, exact
    C = np.asarray(C).astype(np.float32)
    assert x.shape == (B, T) and C.shape == (K, K)
    in_maps = [_host_prep(x[b], C) for b in range(B)]
    res = run_bass_kernel_spmd(
        _get_nc(), in_maps, core_ids=list(range(B)), **(_spmd_kwargs or {})
    )
    out = np.stack([res.results[b]["y"] for b in range(B)], axis=0)
    if _spmd_kwargs:
        kernel.last_results = res
    return out


# revision 6
# speedup vs baseline: 1.4345x; 1.1223x over previous
"""Trainium2 Bass kernel for ConstrainedAttentionModel (sparse_attention).

Full-input contract: kernel(x=[8,2048] int, C=[4,4] f32) -> [8,2048] f32.
Data parallel across 8 NeuronCores: one batch row per core.

Math (per row, T=2048, k=4, V=2048):
  scores[t] = sum_{i,j} C[i,j] * [x[t-j] == x[T-1-i]]   (t-j >= 0)
  scores[T-1] = -inf; attn = softmax(scores)
  out[v] = sum_t attn[t] * [x[t] == v]

v3 design (t = 16p + f layout on 128 partitions):
  - ONE input DMA: the host packs a per-partition image holding the
    fp16 x-window (20 wide), queries replicated across the window,
    C (re-ordered for the conv view), the softmax-mask bias row,
    the base-64 digits of x (lo=x&63, hi=x>>6), the class iotas and
    ones rows
  - windowed equality m[p,i,e] = [x_win[p,e]==q_i], conv view with C
    -> scores; mask folded in as a 17th reduce channel copied from
    the image by the scalar engine
  - exp on the scalar engine in two f-halves with fused row-sum
    accumulation, so the E-weighting of the first half starts early
  - vocab one-hot factorized v = 64*hi + lo in fp16; out[hi,lo] =
    sum_f A_f^T @ B_f as 16 fp16 PSUM-accumulated matmuls
  - sync=False scheduler edges force the DVE score chain ahead of
    the one-hot builds (the greedy list scheduler would otherwise
    interleave them and delay exp by ~1.5us)
  - 1/sum(E) via ones-matmul + reciprocal + ones-broadcast on PE,
    applied by DVE reading PSUM directly
"""
import numpy as np
import concourse.bass as bass
import concourse.bacc as bacc
import concourse.tile as tile
from concourse import mybir
from concourse.tile_rust import add_dep_helper

P = 128
T = 2048
F = T // P  # 16
K = 4
FH = F // 2  # 8
NHI = 32
NLO = 64
WIN = 20  # x-window width per partition (19 used, padded to 20)
NEG = -60000.0  # large-negative mask bias, exactly representable in fp16

fp32 = mybir.dt.float32
fp16 = mybir.dt.float16
i32 = mybir.dt.int32
Alu = mybir.AluOpType
Act = mybir.ActivationFunctionType

# int32-word offsets inside the packed per-partition image
OFF_XW = 0  # [20] fp16 x-window          -> 10 words
OFF_QR = 10  # [4,20] fp16 query replicas   -> 40 words
OFF_CR = 50  # [16] fp16 C (i,jj) order     ->  8 words
OFF_BIAS = 58  # [16] fp16 mask bias row      ->  8 words
OFF_XLO = 66  # [16] fp16 x & 63             ->  8 words
OFF_XHI = 74  # [16] fp16 x >> 6             ->  8 words
OFF_IL = 82  # [64] fp16 iota 0..63         -> 32 words
OFF_IH = 114  # [32] fp16 iota 0..31         -> 16 words
OFF_ONE = 130  # [1] fp32 ones column         ->  1 word
OFF_ONR = 131  # [32] fp32 ones row           -> 32 words
IMG_W = 163

B = 8


def _build_nc():
    nc = bacc.Bacc()
    img = nc.dram_tensor("img", [P, IMG_W], i32, kind="ExternalInput")
    y = nc.dram_tensor("y", [T], fp32, kind="ExternalOutput")

    with tile.TileContext(nc) as tc:
        with (
            tc.tile_pool(name="sb", bufs=1) as sb,
            tc.tile_pool(name="ps", bufs=1, space="PSUM") as ps,
        ):
            IMGT = sb.tile([P, IMG_W], i32)
            nc.sync.dma_start(out=IMGT[:], in_=img[:])

            xw = IMGT[:, OFF_XW : OFF_XW + 10].bitcast(fp16)  # [P, 20]
            qr = IMGT[:, OFF_QR : OFF_QR + 40].bitcast(fp16).rearrange(
                "p (i e) -> p i e", e=WIN
            )  # [P, 4, 20]
            cr = IMGT[:, OFF_CR : OFF_CR + 8].bitcast(fp16).rearrange(
                "p (i jj) -> p i jj", jj=K
            )  # [P, 4, 4]
            bias = IMGT[:, OFF_BIAS : OFF_BIAS + 8].bitcast(fp16)  # [P, 16]
            xlo = IMGT[:, OFF_XLO : OFF_XLO + 8].bitcast(fp16)  # [P, 16]
            xhi = IMGT[:, OFF_XHI : OFF_XHI + 8].bitcast(fp16)  # [P, 16]
            il = IMGT[:, OFF_IL : OFF_IL + 32].bitcast(fp16)  # [P, 64]
            ih = IMGT[:, OFF_IH : OFF_IH + 16].bitcast(fp16)  # [P, 32]
            onec = IMGT[:, OFF_ONE : OFF_ONE + 1].bitcast(fp32)  # [P, 1]
            oner = IMGT[:, OFF_ONR : OFF_ONR + 32].bitcast(fp32)  # [P, 32]

            EQ = sb.tile([P, K, WIN], fp16)  # m[p,i,e] = [xw[p,e]==q_i]
            CE = sb.tile([P, F, 17], fp16)  # c<16: C*m products, c=16: bias
            SC = sb.tile([P, F], fp32)
            E = sb.tile([P, F], fp16)
            RS = sb.tile([P, 2], fp32)
            AEQ = sb.tile([P, F, NHI], fp16)
            BT = sb.tile([P, F, NLO], fp16)
            A = sb.tile([P, F, NHI], fp16)
            SS = sb.tile([1, 1], fp32)
            RINV = sb.tile([1, 1], fp32)
            OUT = sb.tile([NHI, NLO], fp32)
            acc = ps.tile([NHI, NLO], fp32)
            S1 = ps.tile([1, 2], fp32)
            RB = ps.tile([NHI, 1], fp32)

            h0 = slice(0, FH)
            h1 = slice(FH, F)

            # mask bias -> 17th reduce channel (scalar engine, off critical path)
            nc.scalar.activation(out=CE[:, :, 16], in_=bias, func=Act.Copy)

            # ---- score chain (must run first on DVE) ----
            nc.vector.tensor_tensor(
                out=EQ[:],
                in0=xw[:, None, :].broadcast_to([P, K, WIN]),
                in1=qr,
                op=Alu.is_equal,
            )
            eq = EQ[:]
            EQV = bass.AP(
                tensor=eq.tensor,
                offset=eq.offset,
                ap=[eq.ap[0], [1, F], [WIN, K], [1, K]],
            )  # [P, f, i, jj] = m[p, i, f+jj]
            nc.vector.tensor_tensor(
                out=CE[:, :, 0:16].rearrange("p f (i jj) -> p f i jj", jj=K),
                in0=EQV,
                in1=cr[:, None, :, :].broadcast_to([P, F, K, K]),
                op=Alu.mult,
            )
            red = nc.vector.reduce_sum(
                out=SC[:], in_=CE[:], axis=mybir.AxisListType.X
            )
            # E = exp(scores) in halves; RS = per-partition sums
            nc.scalar.activation(
                out=E[:, h0], in_=SC[:, h0], func=Act.Exp,
                accum_out=RS[:, 0:1],
            )
            nc.scalar.activation(
                out=E[:, h1], in_=SC[:, h1], func=Act.Exp,
                accum_out=RS[:, 1:2],
            )
            # S1 = per-half sums over partitions (PE, waits on RS)
            nc.tensor.matmul(
                S1[:], lhsT=onec, rhs=RS[:], start=True, stop=True,
                skip_group_check=True,
            )

            # ---- one-hot builds + weighting + accumulating outer products ----
            def after_scores(bi):
                add_dep_helper(
                    bi.ins, red.ins, sync=False, reason="score chain first"
                )

            a0 = nc.vector.tensor_tensor(
                out=AEQ[:, h0],
                in0=xhi[:, h0][:, :, None].broadcast_to([P, FH, NHI]),
                in1=ih[:, None, :].broadcast_to([P, FH, NHI]),
                op=Alu.is_equal,
            )
            after_scores(a0)
            b0 = nc.vector.tensor_tensor(
                out=BT[:, h0],
                in0=xlo[:, h0][:, :, None].broadcast_to([P, FH, NLO]),
                in1=il[:, None, :].broadcast_to([P, FH, NLO]),
                op=Alu.is_equal,
            )
            after_scores(b0)
            nc.vector.tensor_tensor(
                out=A[:, h0],
                in0=AEQ[:, h0],
                in1=E[:, h0][:, :, None].broadcast_to([P, FH, NHI]),
                op=Alu.mult,
            )
            for f in range(0, FH):
                nc.tensor.matmul(
                    acc[:],
                    lhsT=A[:, f, :],
                    rhs=BT[:, f, :],
                    start=(f == 0),
                    stop=False,
                    skip_group_check=True,
                )
            a1 = nc.vector.tensor_tensor(
                out=AEQ[:, h1],
                in0=xhi[:, h1][:, :, None].broadcast_to([P, FH, NHI]),
                in1=ih[:, None, :].broadcast_to([P, FH, NHI]),
                op=Alu.is_equal,
            )
            after_scores(a1)
            b1 = nc.vector.tensor_tensor(
                out=BT[:, h1],
                in0=xlo[:, h1][:, :, None].broadcast_to([P, FH, NLO]),
                in1=il[:, None, :].broadcast_to([P, FH, NLO]),
                op=Alu.is_equal,
            )
            after_scores(b1)
            nc.vector.tensor_tensor(
                out=A[:, h1],
                in0=AEQ[:, h1],
                in1=E[:, h1][:, :, None].broadcast_to([P, FH, NHI]),
                op=Alu.mult,
            )
            for f in range(FH, F):
                nc.tensor.matmul(
                    acc[:],
                    lhsT=A[:, f, :],
                    rhs=BT[:, f, :],
                    start=False,
                    stop=(f == F - 1),
                    skip_group_check=True,
                )

            # ---- 1/S, scale, store ----
            nc.vector.reduce_sum(out=SS[:], in_=S1[:], axis=mybir.AxisListType.X)
            nc.vector.reciprocal(out=RINV[:], in_=SS[:])
            nc.tensor.matmul(
                RB[:], lhsT=oner[0:1, :], rhs=RINV[:], start=True, stop=True,
                skip_group_check=True,
            )
            nc.vector.tensor_scalar(
                out=OUT[:], in0=acc[:], scalar1=RB[:], scalar2=None, op0=Alu.mult
            )
            nc.sync.dma_start(
                out=y[:].rearrange("(h l) -> h l", l=NLO), in_=OUT[:]
            )
    nc.compile()
    return nc


def _host_prep(x_row: np.ndarray, C: np.ndarray):
    x_row = x_row.astype(np.int32)
    xpad = np.concatenate(
        [np.full(K - 1, -1, np.int32), x_row, np.full(1, -1, np.int32)]
    )
    idx = 16 * np.arange(P)[:, None] + np.arange(WIN)[None, :]
    xw = xpad[idx].astype(np.float16)  # [128, 20]
    q = x_row[T - 1 : T - 1 - K : -1].astype(np.float16)  # q[i] = x[T-1-i]
    qrep = np.tile(q[:, None], (1, WIN)).reshape(-1)  # [80]
    cr = np.ascontiguousarray(C[:, ::-1]).astype(np.float16).reshape(-1)  # [16]
    bias = np.zeros((P, F), np.float16)
    bias[P - 1, F - 1] = NEG
    xt = x_row.reshape(P, F)
    xlo = (xt & 63).astype(np.float16)
    xhi = (xt >> 6).astype(np.float16)
    il = np.arange(NLO, dtype=np.float16)
    ih = np.arange(NHI, dtype=np.float16)
    onec = np.ones(1, np.float32)
    oner = np.ones(NHI, np.float32)

    img = np.empty((P, IMG_W * 4), np.uint8)
    for p in range(P):
        row = np.concatenate(
            [
                xw[p].view(np.uint8),
                qrep.view(np.uint8),
                cr.view(np.uint8),
                bias[p].view(np.uint8),
                xlo[p].view(np.uint8),
                xhi[p].view(np.uint8),
                il.view(np.uint8),
                ih.view(np.uint8),
                onec.view(np.uint8),
                oner.view(np.uint8),
            ]
        )
        img[p] = row
    return {"img": img.view(np.int32)}


_NC_CACHE = {}


def _get_nc():
    if "nc" not in _NC_CACHE:
        _NC_CACHE["nc"] = _build_nc()
    return _NC_CACHE["nc"]


def kernel(x: np.ndarray, C: np.ndarray, _spmd_kwargs: dict | None = None):
    from concourse.bass_utils import run_bass_kernel_spmd

    x = np.asarray(x).astype(np.int32)  # token ids < 2048, exact
    C = np.asarray(C).astype(np.float32)
    assert x.shape == (B, T) and C.shape == (K, K)
    in_maps = [_host_prep(x[b], C) for b in range(B)]
    res = run_bass_kernel_spmd(
        _get_nc(), in_maps, core_ids=list(range(B)), **(_spmd_kwargs or {})
    )
    out = np.stack([res.results[b]["y"] for b in range(B)], axis=0)
    if _spmd_kwargs:
        kernel.last_results = res
    return out
